# revision 65
# baseline (speedup 1.0000x reference)
"""EquivariantAttention Trainium2 kernel.

B=8 batches data-parallel over 8 NeuronCores; per core:
  qkv = x @ W_qkv + b_qkv ; dist = cdist(g, g)
  S^T[j,i] = (q_i.k_j) * exp(-dist)/sqrt(H)   (transposed: j on partitions)
  U^T = exp(S^T); y[i,:] = (U^T)^T @ V' / l_i + b_eff

W_out is folded into the v-projection ON HOST (W_v' = W_v @ W_out,
b_eff = b_v @ W_out + b_out): softmax rows sum to 1 and everything after
the softmax is linear, so y = P @ (x @ W_v') + b_eff exactly. This
deletes the out-projection matmuls, the v-bias ones-matmul trick, and
the 1/l broadcast matmuls from the device kernel (~43k PE cycles).

q,k are stored fp8e4 so the N^2 score matmul runs in DoubleRow perf mode
(K=256 per matmul, half the cycles); 1/sqrt(H) is folded into
E = exp(-dist - ln(sqrt(H))) as an exp bias so q,k keep natural scale.
attn@V' and the v-projection stay bf16: fp8 there fails the 2e-2 budget
(the output is a near-cancelling weighted mean; element rel-errors do
not average down). x^T and W_qkv are bf16 (error contribution ~1e-3).

attn@V' runs in natural (i on partitions) layout: lhsT = U^T i-tile
slices, rhs = V' rows, so psy IS the output chunk; the softmax
normalizer l rides each attn@V group as tiny rhs=ones matmuls into
[P,1] PSUM columns (~zero PE cost), and one fused DVE
scalar_tensor_tensor does (psy * 1/l) + b_eff per i-tile.

E = exp(-dist - ln sqrt(H)) is SYMMETRIC: with three E chunks resident,
every j-tile whose rows lie in a resident earlier chunk is produced as
4 bf16 PE transposes of the already-exp'ed source + one 2x-rate DVE
copy, skipping its d2 matmul, clamp, sqrt AND exp (10 of 16 quads
still computed). The d2 clamp on DVE is mandatory on HW (f32r matmul
noise drives d2 negative near the diagonal; sqrt would NaN; DVE pow
and Pool tensor_scalar both fail walrus codegen, so sqrt stays an ACT
table op) and doubles as the PSUM->SBUF move into the E tile (bf16).

The main loop is software-pipelined across i-chunks: iteration `it`
interleaves, per j-tile step, the DR score matmuls + score*E (DVE) +
expU (ACT) for chunk `it`, the d2/transpose work for chunk `it+1`, and
the natural-layout attn@V + finalize for chunk `it-1`, so PE always
has independent work while DVE drains the score PSUMs. Iteration 0
uses the v-projection as its PE filler; p-state warm-up matmuls keep
the PE clock at 2.4GHz through the prologue, whose x/W DMA stream is
explicitly ordered on the SP queue (the DMA device is serial). The ACT
chain per iteration is [expU x8][sqrt][exp] with the sqrt block
contiguous at the end: exactly two table-set swaps per iteration (the
sqrt placement is latency-critical: anywhere earlier delays the last
expU pairs that the next iteration's attn@V needs; experiments with
mid-iteration placement cost 4-9us).
"""

import numpy as np

import concourse.bass as bass
from concourse import bacc
import concourse.mybir as mybir
import concourse.tile as tile
from concourse.masks import make_identity
from concourse.tile import add_dep_helper

P = 128
H = 512
SC = 512
HT = H // P  # 4

f32 = mybir.dt.float32
f32r = mybir.dt.float32r
bf16 = mybir.dt.bfloat16
f8 = mybir.dt.float8e4
AF = mybir.ActivationFunctionType
OP = mybir.AluOpType
DR = mybir.MatmulPerfMode.DoubleRow
LN_SQRT_H = 0.5 * float(np.log(H))


def _body(tc, n, x, g, wqkv, bqkv, bout, y):
    nc = tc.nc
    NT = n // P
    NC_ = n // SC
    ITC = SC // P  # i-tiles per chunk (4)
    SPB = NT // ITC  # attn@V j-steps per i-tile
    JPS = NT // SPB  # j-tiles per attn@V step

    with (
        nc.allow_low_precision(
            reason="fp8 q/k feed DoubleRow score matmuls; bf16 attn weights"
        ),
        tc.tile_pool(name="const", bufs=1) as const,
        tc.tile_pool(name="geo", bufs=1) as geo,
        tc.tile_pool(name="et_pool", bufs=2) as et_pool,
        tc.tile_pool(name="small", bufs=3) as small,
        tc.tile_pool(name="ps_s", bufs=2, space="PSUM") as ps_s,
        tc.tile_pool(name="ps_d", bufs=2, space="PSUM") as ps_d,
        tc.tile_pool(name="ps_o", bufs=3, space="PSUM") as ps_o,
        tc.tile_pool(name="ps_l", bufs=1, space="PSUM") as ps_l,
    ):
        # ---- constants; DMA order matters: g and the first x group gate
        # the PE pipeline, weights ride the gpsimd queue ----
        bqk_sb = const.tile([P, 8], f32)  # cols 0-3: b_q m-tiles, 4-7: b_k
        bo_bc = const.tile([P, H], f32)  # broadcast b_eff
        ones_bf = const.tile([P, 1], bf16)
        nc.vector.memset(ones_bf, 1.0)
        ebias = const.tile([P, 1], f32)  # exp bias: fold 1/sqrt(H) into E
        nc.vector.memset(ebias, -LN_SQRT_H)

        # augmented geometry, transposed: d2[j,i] = sum_k h_k[j] * g_k[i].
        # f32r, NOT bf16: absolute bf16 error on |g|^2 (~15) is amplified
        # by sqrt near d2=0 and costs ~0.5% output error.
        hT8 = geo.tile([8, n], f32r)
        gT8 = geo.tile([8, n], f32r)

        with (
            tc.tile_pool(name="qkv", bufs=1) as qkv,
            tc.tile_pool(name="e_pool", bufs=3) as e_pool,
            tc.tile_pool(name="ut_pool", bufs=2) as ut_pool,
            tc.tile_pool(name="xt_pool", bufs=1) as xt_pool,
        ):
            qT = qkv.tile([P, HT, n], f8)  # q^T (natural scale), [h, i]
            kT = qkv.tile([P, HT, n], f8)  # k^T, [h, j]
            v_bf = qkv.tile([P, NT, H], bf16)  # v' natural, [j, h]
            wqkv_bf = qkv.tile([P, HT, 3 * H], bf16)
            xT = xt_pool.tile([P, HT, n], bf16)
            # f32 identity: a bf16 identity with f32(r) data crashes the
            # exec unit on HW (mixed-dtype transpose), don't try it
            ident = xt_pool.tile([P, P], f32)
            make_identity(nc, ident)
            # f32r twin for the x transposes: the BIR verifier requires
            # f32r matmul operands to be PRODUCED as f32r (ACT copy or
            # DMA), a bitcast view of f32-written memory is rejected
            ident_r = xt_pool.tile([P, P], f32r)
            nc.scalar.copy(ident_r, ident)
            ident_bf = xt_pool.tile([P, P], bf16)
            nc.scalar.copy(ident_bf, ident)

            # ---- ACT chain helper: keeps sqrt/exp table-set switches at
            # two per iteration by pinning ACT emission order ----
            state = {"prev": None}

            def chain(a):
                if state["prev"] is not None:
                    add_dep_helper(
                        a.ins,
                        state["prev"].ins,
                        sync=False,
                        reason="ACT table-set batching",
                    )
                state["prev"] = a
                return a

            Es = {}
            SYM = NT == 16 and NC_ == 4

            def emit_d2(ic, jt):
                # one d2 matmul + DVE clamp into the E tile (bf16). The
                # clamp is mandatory on HW (f32r noise -> negative d2 ->
                # sqrt NaN) and doubles as the PSUM->SBUF move.
                isl = slice(ic * SC, (ic + 1) * SC)
                if ic not in Es:
                    Etile = e_pool.tile([P, NT, SC], bf16, tag="E")
                    Es[ic] = Etile
                dist = Es[ic]
                psd = ps_d.tile([P, SC], f32, tag="psd")
                nc.tensor.matmul(
                    psd,
                    lhsT=hT8[:, jt * P : (jt + 1) * P],
                    rhs=gT8[:, isl],
                    start=True,
                    stop=True,
                )
                nc.vector.tensor_scalar_max(dist[:, jt, :], psd, 0.0)

            def emit_eT(nxt, jt, Ec):
                # E is symmetric: the j-tiles of chunk nxt whose rows lie
                # in chunk cur are bf16 TRANSPOSES of the already-exp'ed
                # E(cur) -- skips their d2 matmul, clamp, sqrt and exp.
                # 4 PE transposes pack one [P,SC] bf16 PSUM row evacuated
                # by a single 2x-rate DVE copy.
                rc = jt // ITC  # row-chunk this j-tile belongs to
                if nxt not in Es:
                    Etile = e_pool.tile([P, NT, SC], bf16, tag="E")
                    Es[nxt] = Etile
                En = Es[nxt]
                Esrc = Ec if rc == nxt - 1 else Es[rc]
                psd = ps_d.tile([P, SC], f32, tag="psd")
                pbf = psd.bitcast(bf16)
                for t in range(ITC):
                    nc.tensor.transpose(
                        pbf[:, t * P : (t + 1) * P],
                        Esrc[
                            :,
                            ITC * nxt + t,
                            (jt - ITC * rc) * P : (jt - ITC * rc + 1) * P,
                        ],
                        ident_bf,
                    )
                nc.vector.tensor_copy(En[:, jt, :], pbf[:, : ITC * P])

            def emit_sqrt_exp(ic):
                # in-place on the E tile: sqrt then exp with the
                # -ln(sqrt(H)) bias folded in. DVE pow and Pool
                # tensor_scalar both fail walrus codegen, so the sqrt
                # must be ACT table ops; the chain keeps the sqrt block
                # contiguous so each iteration pays exactly two
                # sqrt<->exp table-set swaps.
                E = Es[ic]
                if SYM and ic >= 1:
                    # quads transposable from a resident chunk (ring
                    # distance <= 2) are skipped; the rest get sqrt+exp
                    spans = [
                        (4 * q, 4)
                        for q in range(ITC)
                        if not (q < ic and ic - q <= 2)
                    ]
                else:
                    bq = min(8, NT)
                    spans = [(jp, bq) for jp in range(0, NT, bq)]
                for jp, w in spans:
                    chain(
                        nc.scalar.activation(
                            E[:, jp : jp + w, :], E[:, jp : jp + w, :],
                            AF.Sqrt,
                        )
                    )
                for jp, w in spans:
                    chain(
                        nc.scalar.activation(
                            E[:, jp : jp + w, :],
                            E[:, jp : jp + w, :],
                            AF.Exp,
                            scale=-1.0,
                            bias=ebias,
                        )
                    )

            # ---- prologue: geometry prep, x transposes, W staging ----
            with tc.tile_pool(name="wstage", bufs=1) as wstage:
                g_sb = wstage.tile([P, NT, 3], f32)
                # The DMA engine pool is effectively serial: x0 must be the
                # very first sizable transfer so PE starts transposing at
                # ~2.5us; W_qkv rides in quarters interleaved between x
                # groups (each quarter converts to bf16 on Pool while the
                # next x group transfers); constants trail the x stream.
                # q/k projection blocks are emitted INSIDE the x loop as
                # soon as their x-chunk and W-quarter have been requested,
                # so PE computes behind the serial DMA stream instead of
                # idling until everything lands.
                wq_r = wqkv.rearrange("(kt p) m -> p kt m", p=P)
                WQ = 3 * H // 4
                NH = max(1, NT // 8)
                NG = NT // NH
                # SYM: chunk 1's computed (non-transposable) d2 j-tiles
                # also ride the prologue, where DVE has ~12us of slack
                # that iteration 0 (mult+clamp+v-copy loaded) does not
                d2q = [(0, j) for j in range(NT)]

                def emit_w_quarter(wh):
                    # DMA rides the SP queue (with the rest of the ordered
                    # prologue stream); the bf16 convert stays on Pool
                    msl = slice(wh * WQ, (wh + 1) * WQ)
                    w_q = wstage.tile([P, HT, WQ], f32, tag=f"wq{wh % 2}")
                    nc.sync.dma_start(w_q, wq_r[:, :, msl])
                    nc.gpsimd.tensor_copy(wqkv_bf[:, :, msl], w_q)

                def emit_qk_block(c, mt):
                    dst = qT if mt < 4 else kT
                    mi = mt % 4
                    ps = ps_s.tile([P, SC], f32, tag="pss")
                    for kc in range(HT):
                        nc.tensor.matmul(
                            ps,
                            lhsT=wqkv_bf[:, kc, mt * P : (mt + 1) * P],
                            rhs=xT[:, kc, c * SC : (c + 1) * SC],
                            start=(kc == 0),
                            stop=(kc == HT - 1),
                        )
                    if d2q and (mt + c) % 2 == 1:
                        emit_d2(*d2q.pop(0))
                    dap = dst[:, mi, c * SC : (c + 1) * SC]
                    if (mt + c) % 2 == 0:
                        nc.scalar.activation(
                            dap, ps, AF.Identity, bias=bqk_sb[:, mt : mt + 1]
                        )
                    else:
                        nc.vector.tensor_scalar_add(
                            dap, ps, bqk_sb[:, mt : mt + 1]
                        )

                # static schedule: after group qi, which W quarters to
                # request and which (c, mt) blocks are fully fed.
                # x-chunk c completes with group (4*(c+1))//NH - 1;
                # W quarter wh is requested after group 2*wh+1 (or at the
                # end) and q/k block (c, mt) needs quarter mt//3.
                wq_after = {}
                for wh in range(4):
                    gidx = wh + 1 if wh + 1 < NG else NG - 1
                    wq_after.setdefault(gidx, []).append(wh)
                blk_after = {}
                for c in range(NC_):
                    gx = (4 * (c + 1)) // NH - 1
                    for mt in range(8):
                        wh = min(mt // 3, 3)
                        gw = wh + 1 if wh + 1 < NG else NG - 1
                        blk_after.setdefault(max(gx, gw), []).append((c, mt))

                x_r = x.rearrange("(nt p) h -> p nt h", p=P)
                with tc.tile_pool(name="xsb_pool", bufs=2) as xsb_pool:
                    # g rides the SP queue FIRST on the serial DMA device:
                    # geometry prep + transposes + the first d2 matmuls
                    # give PE ~7us of work that needs only g, hiding the
                    # x0 transfer latency that used to stall startup.
                    nc.sync.dma_start(
                        g_sb, g.rearrange("(nt p) c -> p nt c", p=P)
                    )
                    nc.sync.dma_start(
                        bqk_sb,
                        bqkv[0 : 2 * H].rearrange("(mt p) -> p mt", p=P),
                    )
                    # p-state warm-up: the PE only reaches 2.4GHz after
                    # ~3us of CONTINUOUS execution; a few dependency-free
                    # dummy matmuls during the otherwise-idle g/x DMA
                    # window mean the real prologue starts at full clock
                    dumw = wstage.tile([P, SC], f32r)
                    nc.vector.memset(dumw.bitcast(f32), 0.0)
                    for _ in range(7):
                        psw = ps_o.tile([P, SC], f32, tag="pso")
                        nc.tensor.matmul(
                            psw,
                            lhsT=dumw[:, 0:P],
                            rhs=dumw,
                            start=True,
                            stop=True,
                        )
                    g2 = wstage.tile([P, NT, 3], f32)
                    nc.vector.tensor_mul(g2, g_sb, g_sb)
                    sq = wstage.tile([P, NT, 1], f32)
                    nc.vector.reduce_sum(sq, g2, axis=mybir.AxisListType.X)
                    Ag = wstage.tile([P, NT, 8], f32)
                    Ah = wstage.tile([P, NT, 8], f32)
                    nc.vector.memset(Ag, 0.0)
                    nc.vector.memset(Ah, 0.0)
                    nc.vector.tensor_copy(Ag[:, :, 0:3], g_sb)
                    nc.vector.tensor_copy(Ag[:, :, 3:4], sq)
                    nc.vector.memset(Ag[:, :, 4:5], 1.0)
                    nc.vector.tensor_scalar_mul(Ah[:, :, 0:3], g_sb, -2.0)
                    nc.vector.memset(Ah[:, :, 3:4], 1.0)
                    nc.vector.tensor_copy(Ah[:, :, 4:5], sq)
                    for nt in range(NT):
                        pt = ps_s.tile([P, SC], f32, tag="pss")
                        nc.tensor.transpose(pt[:8, :P], Ah[:, nt, :], ident)
                        nc.scalar.copy(
                            hT8[:, nt * P : (nt + 1) * P], pt[:8, :P]
                        )
                        pt2 = ps_d.tile([P, SC], f32, tag="psd")
                        nc.tensor.transpose(pt2[:8, :P], Ag[:, nt, :], ident)
                        nc.scalar.copy(
                            gT8[:, nt * P : (nt + 1) * P], pt2[:8, :P]
                        )
                    for _ in range(4):
                        emit_d2(*d2q.pop(0))
                    for qi, hh in enumerate(range(0, NT, NH)):
                        x_sb = xsb_pool.tile([P, NH, H], f32, tag="x_sb")
                        nc.sync.dma_start(x_sb, x_r[:, hh : hh + NH, :])
                        # Pool converts the group to bf16 (SBUF->SBUF);
                        # bf16 transposes run at 1 cycle/row and 4 of
                        # them pack one [P,SC] bf16 PSUM row evacuated by
                        # a single 2x-rate DVE copy into the strided xT
                        # destination
                        x_bf = xsb_pool.tile([P, NH, H], bf16, tag="x_bf")
                        if qi == 0:
                            # idle DVE converts the first group in halves
                            # so transposing starts ~1.5us sooner than
                            # waiting on the Pool engine
                            for nt in range(NH):
                                nc.vector.tensor_copy(
                                    x_bf[:, nt, :], x_sb[:, nt, :]
                                )
                        else:
                            nc.gpsimd.tensor_copy(x_bf, x_sb)
                        for wh in wq_after.get(qi, []):
                            emit_w_quarter(wh)
                        for nt in range(NH):
                            pt = ps_o.tile([P, SC], f32, tag="pso")
                            pbf = pt.bitcast(bf16)
                            for ht in range(HT):
                                nc.tensor.transpose(
                                    pbf[:, ht * P : (ht + 1) * P],
                                    x_bf[:, nt, ht * P : (ht + 1) * P],
                                    ident_bf,
                                )
                            nc.vector.tensor_copy(
                                xT[:, :, (hh + nt) * P : (hh + nt + 1) * P],
                                pbf[:, : HT * P],
                            )
                        for c, mt in blk_after.get(qi, []):
                            emit_qk_block(c, mt)

                nc.gpsimd.dma_start(bo_bc, bout.partition_broadcast(P))

            while d2q:
                emit_d2(*d2q.pop(0))
            emit_sqrt_exp(0)

            # ---- pipelined main loop over i-chunks ----
            y_r = y.rearrange("(nt p) h -> p nt h", p=P)
            UTs = {}
            # one persistent bank for the row-sum columns, halves
            # alternating by iteration parity (slice-level deps avoid a
            # cross-iteration WAR stall without a second bank)
            psl2 = ps_l.tile([P, 2, ITC], f32, tag="psl")

            for it in range(NC_ + 1):
                cur, prv, nxt = it, it - 1, it + 1
                E = Es.get(cur) if cur < NC_ else None
                if cur < NC_:
                    UTc = ut_pool.tile([P, NT, SC], bf16, tag="UT")
                    UTs[cur] = UTc
                    isl = slice(cur * SC, (cur + 1) * SC)
                if prv >= 0:
                    UTp = UTs.pop(prv)
                    # softmax row-sums for prv, in column form: tiny
                    # rhs=ones matmuls ride each attn@V group (same lhsT
                    # slices), so l_i lands as [P,1] PSUM columns with
                    # ~zero PE occupancy and no ACT wait; a per-column
                    # reciprocal feeds the fused finalize directly.
                    psl_c = psl2[:, prv % 2, :]
                    lc = et_pool.tile([P, ITC], f32, tag="lc")
                for jt in range(NT):
                    if cur < NC_:
                        jo = jt
                        jsl = slice(jo * P, (jo + 1) * P)
                        pss = ps_s.tile([P, SC], f32, tag="pss")
                        for kc in (0, 2):
                            nc.tensor.matmul(
                                pss,
                                lhsT=kT[:, kc : kc + 2, jsl],
                                rhs=qT[:, kc : kc + 2, isl],
                                start=(kc == 0),
                                stop=(kc == 2),
                                perf_mode=DR,
                            )
                        # score*E + expU right after QK: this is the
                        # iteration's critical path, so it goes first in
                        # the in-order DVE queue (clamps are not urgent
                        # and follow). NOT paired: the evacuation must
                        # start as soon as half jt lands or the scores
                        # pipeline stalls on the 2-bank ring.
                        if jt % 2 == 0:
                            et2 = et_pool.tile([P, 2, SC], f32, tag="et")
                        nc.vector.tensor_mul(
                            et2[:, jt % 2, :], pss, E[:, jo, :]
                        )
                        if jt % 2 == 1:
                            chain(
                                nc.scalar.activation(
                                    UTc[:, jo - 1 : jo + 1, :], et2, AF.Exp
                                )
                            )
                    if it == 0:
                        # v' natural [j, h] (bf16) as iteration-0 PE filler
                        psv = ps_o.tile([P, SC], f32, tag="pso")
                        for kc in range(HT):
                            nc.tensor.matmul(
                                psv,
                                lhsT=xT[:, kc, jt * P : (jt + 1) * P],
                                rhs=wqkv_bf[:, kc, 2 * H : 3 * H],
                                start=(kc == 0),
                                stop=(kc == HT - 1),
                            )
                        if jt % 2 == 0:
                            nc.scalar.copy(v_bf[:, jt, :], psv)
                        else:
                            nc.vector.tensor_copy(v_bf[:, jt, :], psv)
                    if prv >= 0:
                        # attn@V for prv in natural layout: i-tile-major,
                        # psy accumulates the output chunk directly
                        it4, sub = jt // SPB, jt % SPB
                        if sub == 0:
                            psy = ps_o.tile([P, SC], f32, tag="pso")
                        for jtt in range(sub * JPS, (sub + 1) * JPS):
                            ut_ap = UTp[:, jtt, it4 * P : (it4 + 1) * P]
                            nc.tensor.matmul(
                                psy,
                                lhsT=ut_ap,
                                rhs=v_bf[:, jtt, :],
                                start=(jtt == 0),
                                stop=(jtt == NT - 1),
                            )
                            nc.tensor.matmul(
                                psl_c[:, it4 : it4 + 1],
                                lhsT=ut_ap,
                                rhs=ones_bf,
                                start=(jtt == 0),
                                stop=(jtt == NT - 1),
                            )
                        if sub == SPB - 1:
                            # single fused finalize on DVE: with the sqrt
                            # back on ACT, ACT is the loaded engine and
                            # DVE has the slack
                            nc.vector.reciprocal(
                                lc[:, it4 : it4 + 1], psl_c[:, it4 : it4 + 1]
                            )
                            ysb = small.tile([P, H], f32, tag="ysb")
                            nc.vector.scalar_tensor_tensor(
                                ysb,
                                psy,
                                lc[:, it4 : it4 + 1],
                                bo_bc,
                                OP.mult,
                                OP.add,
                            )
                            nc.sync.dma_start(
                                y_r[:, prv * ITC + it4, :], ysb
                            )
                    if nxt < NC_:
                        # schedule the 4 symmetric-transpose j-tiles at
                        # the END steps: their PE transposes read the
                        # exp'ed E(cur), which the in-order ACT queue may
                        # still be finishing early in the iteration
                        if SYM:
                            # j-tiles whose rows lie in a RESIDENT earlier
                            # chunk (ring distance <= 2) are bf16
                            # transposes; the rest need the d2 pipeline.
                            # The contiguous sqrt+exp ACT block fires as
                            # soon as the last computed quad's clamp is
                            # in flight (same 2 table swaps anywhere in
                            # the iteration, but landing early unblocks
                            # the next iteration's first score*E mults).
                            eTjs = [
                                j for j in range(NT)
                                if j // ITC < nxt and nxt - j // ITC <= 2
                            ]
                            d2js = [j for j in range(NT) if j not in eTjs]
                            if jt < len(d2js):
                                emit_d2(nxt, d2js[jt])
                            elif jt >= NT - len(eTjs):
                                emit_eT(nxt, eTjs[jt - (NT - len(eTjs))], E)
                        else:
                            emit_d2(nxt, jt)
                if nxt < NC_:
                    emit_sqrt_exp(nxt)


def build_bass(n: int = 2048) -> bass.Bass:
    nc = bacc.Bacc(None, target_bir_lowering=False)
    x = nc.dram_tensor("x", [n, H], f32, kind="ExternalInput")[:, :]
    g = nc.dram_tensor("g", [n, 3], f32, kind="ExternalInput")[:, :]
    wqkv = nc.dram_tensor("w_qkv", [H, 3 * H], f32, kind="ExternalInput")[:, :]
    bqkv = nc.dram_tensor("b_qkv", [3 * H], f32, kind="ExternalInput")[:]
    bout = nc.dram_tensor("b_out", [H], f32, kind="ExternalInput")[:]
    y = nc.dram_tensor("y", [n, H], f32, kind="ExternalOutput")[:, :]
    with tile.TileContext(nc) as tc:
        _body(tc, n, x, g, wqkv, bqkv, bout, y)
    nc.finalize()
    return nc


_CACHED = {}


def _get_nc(n: int = 2048) -> bass.Bass:
    if n not in _CACHED:
        _CACHED[n] = build_bass(n)
    return _CACHED[n]


def _fold_wout(wqkv, bqkv, wout, bout):
    """Host-side fold of W_out into the v-projection (exact: the model
    is linear after the softmax and softmax rows sum to 1)."""
    wqkv_eff = np.array(wqkv, dtype=np.float32, copy=True)
    wv = wqkv[:, 2 * H :].astype(np.float64)
    wqkv_eff[:, 2 * H :] = (wv @ wout.astype(np.float64)).astype(np.float32)
    beff = (
        bqkv[2 * H :].astype(np.float64) @ wout.astype(np.float64)
        + bout.astype(np.float64)
    ).astype(np.float32)
    return wqkv_eff, beff


def kernel(**inputs) -> np.ndarray:
    from concourse.bass_utils import run_bass_kernel_spmd

    x = np.ascontiguousarray(inputs["x"], dtype=np.float32)
    g = np.ascontiguousarray(inputs["geometric_features"], dtype=np.float32)
    wqkv = np.ascontiguousarray(inputs["W_qkv"], dtype=np.float32)
    bqkv = np.ascontiguousarray(inputs["b_qkv"], dtype=np.float32)
    wout = np.ascontiguousarray(inputs["W_out"], dtype=np.float32)
    bout = np.ascontiguousarray(inputs["b_out"], dtype=np.float32)

    wqkv_eff, beff = _fold_wout(wqkv, bqkv, wout, bout)

    B, n, _ = x.shape
    nc = _get_nc(n)
    core_ids = list(range(B))
    in_maps = [
        {
            "x": np.ascontiguousarray(x[b]),
            "g": np.ascontiguousarray(g[b]),
            "w_qkv": wqkv_eff,
            "b_qkv": bqkv,
            "b_out": beff,
        }
        for b in range(B)
    ]
    res = run_bass_kernel_spmd(nc, in_maps, core_ids)
    return np.stack([res.results[b]["y"] for b in range(B)]).astype(np.float32)


# revision 66
# speedup vs baseline: 1.0187x; 1.0187x over previous
"""EquivariantAttention Trainium2 kernel.

B=8 batches data-parallel over 8 NeuronCores; per core:
  qkv = x @ W_qkv + b_qkv ; dist = cdist(g, g)
  S^T[j,i] = (q_i.k_j) * exp(-dist)/sqrt(H)   (transposed: j on partitions)
  U^T = exp(S^T); y[i,:] = (U^T)^T @ V' / l_i + b_eff

W_out is folded into the v-projection ON HOST (W_v' = W_v @ W_out,
b_eff = b_v @ W_out + b_out): softmax rows sum to 1 and everything after
the softmax is linear, so y = P @ (x @ W_v') + b_eff exactly. This
deletes the out-projection matmuls, the v-bias ones-matmul trick, and
the 1/l broadcast matmuls from the device kernel (~43k PE cycles).

q,k are stored fp8e4 so the N^2 score matmul runs in DoubleRow perf mode
(K=256 per matmul, half the cycles); 1/sqrt(H) is folded into
E = exp(-dist - ln(sqrt(H))) as an exp bias so q,k keep natural scale.
attn@V' and the v-projection stay bf16: fp8 there fails the 2e-2 budget
(the output is a near-cancelling weighted mean; element rel-errors do
not average down). x^T and W_qkv are bf16 (error contribution ~1e-3).

attn@V' runs in natural (i on partitions) layout: lhsT = U^T i-tile
slices, rhs = V' rows, so psy IS the output chunk; the softmax
normalizer l rides each attn@V group as tiny rhs=ones matmuls into
[P,1] PSUM columns (~zero PE cost), and one fused DVE
scalar_tensor_tensor does (psy * 1/l) + b_eff per i-tile.

E = exp(-dist - ln sqrt(H)) is SYMMETRIC: with three E chunks resident,
every j-tile whose rows lie in a resident earlier chunk is produced as
4 bf16 PE transposes of the already-exp'ed source + one 2x-rate DVE
copy, skipping its d2 matmul, clamp, sqrt AND exp (10 of 16 quads
still computed). The d2 clamp on DVE is mandatory on HW (f32r matmul
noise drives d2 negative near the diagonal; sqrt would NaN; DVE pow
and Pool tensor_scalar both fail walrus codegen, so sqrt stays an ACT
table op) and doubles as the PSUM->SBUF move into the E tile (bf16).

The main loop is software-pipelined across i-chunks: iteration `it`
interleaves, per j-tile step, the DR score matmuls + score*E (DVE) +
expU (ACT) for chunk `it`, the d2/transpose work for chunk `it+1`, and
the natural-layout attn@V + finalize for chunk `it-1`, so PE always
has independent work while DVE drains the score PSUMs. Iteration 0
uses the v-projection as its PE filler; p-state warm-up matmuls keep
the PE clock at 2.4GHz through the prologue, whose x/W DMA stream is
explicitly ordered on the SP queue (the DMA device is serial). The ACT
chain per iteration is [expU x8][sqrt][exp] with the sqrt block
contiguous at the end: exactly two table-set swaps per iteration (the
sqrt placement is latency-critical: anywhere earlier delays the last
expU pairs that the next iteration's attn@V needs; experiments with
mid-iteration placement cost 4-9us).
"""

import numpy as np

import concourse.bass as bass
from concourse import bacc
import concourse.mybir as mybir
import concourse.tile as tile
from concourse.masks import make_identity
from concourse.tile import add_dep_helper

P = 128
H = 512
SC = 512
HT = H // P  # 4

f32 = mybir.dt.float32
f32r = mybir.dt.float32r
bf16 = mybir.dt.bfloat16
f8 = mybir.dt.float8e4
AF = mybir.ActivationFunctionType
OP = mybir.AluOpType
DR = mybir.MatmulPerfMode.DoubleRow
LN_SQRT_H = 0.5 * float(np.log(H))


def _body(tc, n, x, g, wqkv, bqkv, bout, y):
    nc = tc.nc
    NT = n // P
    NC_ = n // SC
    ITC = SC // P  # i-tiles per chunk (4)
    SPB = NT // ITC  # attn@V j-steps per i-tile
    JPS = NT // SPB  # j-tiles per attn@V step

    with (
        nc.allow_low_precision(
            reason="fp8 q/k feed DoubleRow score matmuls; bf16 attn weights"
        ),
        tc.tile_pool(name="const", bufs=1) as const,
        tc.tile_pool(name="geo", bufs=1) as geo,
        tc.tile_pool(name="et_pool", bufs=2) as et_pool,
        tc.tile_pool(name="small", bufs=3) as small,
        tc.tile_pool(name="ps_s", bufs=2, space="PSUM") as ps_s,
        tc.tile_pool(name="ps_d", bufs=2, space="PSUM") as ps_d,
        tc.tile_pool(name="ps_o", bufs=3, space="PSUM") as ps_o,
        tc.tile_pool(name="ps_l", bufs=1, space="PSUM") as ps_l,
    ):
        # ---- constants; DMA order matters: g and the first x group gate
        # the PE pipeline, weights ride the gpsimd queue ----
        bqk_sb = const.tile([P, 8], f32)  # cols 0-3: b_q m-tiles, 4-7: b_k
        bo_bc = const.tile([P, H], f32)  # broadcast b_eff
        ones_bf = const.tile([P, 1], bf16)
        nc.vector.memset(ones_bf, 1.0)
        ebias = const.tile([P, 1], f32)  # exp bias: fold 1/sqrt(H) into E
        nc.vector.memset(ebias, -LN_SQRT_H)

        # augmented geometry, transposed: d2[j,i] = sum_k h_k[j] * g_k[i].
        # f32r, NOT bf16: absolute bf16 error on |g|^2 (~15) is amplified
        # by sqrt near d2=0 and costs ~0.5% output error.
        hT8 = geo.tile([8, n], f32r)
        gT8 = geo.tile([8, n], f32r)

        with (
            tc.tile_pool(name="qkv", bufs=1) as qkv,
            tc.tile_pool(name="e_pool", bufs=3) as e_pool,
            tc.tile_pool(name="ut_pool", bufs=2) as ut_pool,
            tc.tile_pool(name="xt_pool", bufs=1) as xt_pool,
        ):
            qT = qkv.tile([P, HT, n], f8)  # q^T (natural scale), [h, i]
            kT = qkv.tile([P, HT, n], f8)  # k^T, [h, j]
            v_bf = qkv.tile([P, NT, H], bf16)  # v' natural, [j, h]
            wqkv_bf = qkv.tile([P, HT, 3 * H], bf16)
            xT = xt_pool.tile([P, HT, n], bf16)
            # f32 identity: a bf16 identity with f32(r) data crashes the
            # exec unit on HW (mixed-dtype transpose), don't try it
            ident = xt_pool.tile([P, P], f32)
            make_identity(nc, ident)
            # f32r twin for the x transposes: the BIR verifier requires
            # f32r matmul operands to be PRODUCED as f32r (ACT copy or
            # DMA), a bitcast view of f32-written memory is rejected
            ident_r = xt_pool.tile([P, P], f32r)
            nc.scalar.copy(ident_r, ident)
            ident_bf = xt_pool.tile([P, P], bf16)
            nc.scalar.copy(ident_bf, ident)

            # ---- ACT chain helper: keeps sqrt/exp table-set switches at
            # two per iteration by pinning ACT emission order ----
            state = {"prev": None}

            def chain(a):
                if state["prev"] is not None:
                    add_dep_helper(
                        a.ins,
                        state["prev"].ins,
                        sync=False,
                        reason="ACT table-set batching",
                    )
                state["prev"] = a
                return a

            Es = {}
            SYM = NT == 16 and NC_ == 4

            def emit_d2(ic, jt):
                # one d2 matmul + DVE clamp into the E tile (bf16). The
                # clamp is mandatory on HW (f32r noise -> negative d2 ->
                # sqrt NaN) and doubles as the PSUM->SBUF move.
                isl = slice(ic * SC, (ic + 1) * SC)
                if ic not in Es:
                    Etile = e_pool.tile([P, NT, SC], bf16, tag="E")
                    Es[ic] = Etile
                dist = Es[ic]
                psd = ps_d.tile([P, SC], f32, tag="psd")
                nc.tensor.matmul(
                    psd,
                    lhsT=hT8[:, jt * P : (jt + 1) * P],
                    rhs=gT8[:, isl],
                    start=True,
                    stop=True,
                )
                nc.vector.tensor_scalar_max(dist[:, jt, :], psd, 0.0)

            def emit_eT(nxt, jt, Ec):
                # E is symmetric: the j-tiles of chunk nxt whose rows lie
                # in chunk cur are bf16 TRANSPOSES of the already-exp'ed
                # E(cur) -- skips their d2 matmul, clamp, sqrt and exp.
                # 4 PE transposes pack one [P,SC] bf16 PSUM row evacuated
                # by a single 2x-rate DVE copy.
                rc = jt // ITC  # row-chunk this j-tile belongs to
                if nxt not in Es:
                    Etile = e_pool.tile([P, NT, SC], bf16, tag="E")
                    Es[nxt] = Etile
                En = Es[nxt]
                Esrc = Ec if rc == nxt - 1 else Es[rc]
                psd = ps_d.tile([P, SC], f32, tag="psd")
                pbf = psd.bitcast(bf16)
                for t in range(ITC):
                    nc.tensor.transpose(
                        pbf[:, t * P : (t + 1) * P],
                        Esrc[
                            :,
                            ITC * nxt + t,
                            (jt - ITC * rc) * P : (jt - ITC * rc + 1) * P,
                        ],
                        ident_bf,
                    )
                nc.vector.tensor_copy(En[:, jt, :], pbf[:, : ITC * P])

            def emit_sqrt_exp(ic):
                # in-place on the E tile: sqrt then exp with the
                # -ln(sqrt(H)) bias folded in. DVE pow and Pool
                # tensor_scalar both fail walrus codegen, so the sqrt
                # must be ACT table ops; the chain keeps the sqrt block
                # contiguous so each iteration pays exactly two
                # sqrt<->exp table-set swaps.
                E = Es[ic]
                if SYM and ic >= 1:
                    # quads transposable from a resident chunk (ring
                    # distance <= 2) are skipped; the rest get sqrt+exp
                    spans = [
                        (4 * q, 4)
                        for q in range(ITC)
                        if not (q < ic and ic - q <= 2)
                    ]
                else:
                    bq = min(8, NT)
                    spans = [(jp, bq) for jp in range(0, NT, bq)]
                for jp, w in spans:
                    chain(
                        nc.scalar.activation(
                            E[:, jp : jp + w, :], E[:, jp : jp + w, :],
                            AF.Sqrt,
                        )
                    )
                for jp, w in spans:
                    chain(
                        nc.scalar.activation(
                            E[:, jp : jp + w, :],
                            E[:, jp : jp + w, :],
                            AF.Exp,
                            scale=-1.0,
                            bias=ebias,
                        )
                    )

            # ---- prologue: geometry prep, x transposes, W staging ----
            with tc.tile_pool(name="wstage", bufs=1) as wstage:
                g_sb = wstage.tile([P, NT, 3], f32)
                # The DMA engine pool is effectively serial: x0 must be the
                # very first sizable transfer so PE starts transposing at
                # ~2.5us; W_qkv rides in quarters interleaved between x
                # groups (each quarter converts to bf16 on Pool while the
                # next x group transfers); constants trail the x stream.
                # q/k projection blocks are emitted INSIDE the x loop as
                # soon as their x-chunk and W-quarter have been requested,
                # so PE computes behind the serial DMA stream instead of
                # idling until everything lands.
                wq_r = wqkv.rearrange("(kt p) m -> p kt m", p=P)
                WQ = 3 * H // 4
                NH = max(1, NT // 8)
                NG = NT // NH
                # SYM: chunk 1's computed (non-transposable) d2 j-tiles
                # also ride the prologue, where DVE has ~12us of slack
                # that iteration 0 (mult+clamp+v-copy loaded) does not
                # SYM: chunk 1's computed (non-transposable) d2 j-tiles
                # also ride the prologue, whose DVE has slack after the
                # bf16-x change; iteration 0 is otherwise DVE-bound on
                # mult+clamp+v-copy evacuations
                d2q = [(0, j) for j in range(NT)]
                if NT == 16 and NC_ == 4:
                    d2q += [(1, j) for j in range(ITC, NT)]

                def emit_w_quarter(wh):
                    # DMA rides the SP queue (with the rest of the ordered
                    # prologue stream); the bf16 convert stays on Pool
                    msl = slice(wh * WQ, (wh + 1) * WQ)
                    w_q = wstage.tile([P, HT, WQ], f32, tag=f"wq{wh % 2}")
                    nc.sync.dma_start(w_q, wq_r[:, :, msl])
                    nc.gpsimd.tensor_copy(wqkv_bf[:, :, msl], w_q)

                def emit_qk_block(c, mt):
                    dst = qT if mt < 4 else kT
                    mi = mt % 4
                    ps = ps_s.tile([P, SC], f32, tag="pss")
                    for kc in range(HT):
                        nc.tensor.matmul(
                            ps,
                            lhsT=wqkv_bf[:, kc, mt * P : (mt + 1) * P],
                            rhs=xT[:, kc, c * SC : (c + 1) * SC],
                            start=(kc == 0),
                            stop=(kc == HT - 1),
                        )
                    if d2q and (mt + c) % 2 == 1:
                        emit_d2(*d2q.pop(0))
                    dap = dst[:, mi, c * SC : (c + 1) * SC]
                    if (mt + c) % 2 == 0:
                        nc.scalar.activation(
                            dap, ps, AF.Identity, bias=bqk_sb[:, mt : mt + 1]
                        )
                    else:
                        nc.vector.tensor_scalar_add(
                            dap, ps, bqk_sb[:, mt : mt + 1]
                        )

                # static schedule: after group qi, which W quarters to
                # request and which (c, mt) blocks are fully fed.
                # x-chunk c completes with group (4*(c+1))//NH - 1;
                # W quarter wh is requested after group 2*wh+1 (or at the
                # end) and q/k block (c, mt) needs quarter mt//3.
                wq_after = {}
                for wh in range(4):
                    gidx = wh + 1 if wh + 1 < NG else NG - 1
                    wq_after.setdefault(gidx, []).append(wh)
                blk_after = {}
                for c in range(NC_):
                    gx = (4 * (c + 1)) // NH - 1
                    for mt in range(8):
                        wh = min(mt // 3, 3)
                        gw = wh + 1 if wh + 1 < NG else NG - 1
                        blk_after.setdefault(max(gx, gw), []).append((c, mt))

                x_r = x.rearrange("(nt p) h -> p nt h", p=P)
                with tc.tile_pool(name="xsb_pool", bufs=2) as xsb_pool:
                    # g rides the SP queue FIRST on the serial DMA device:
                    # geometry prep + transposes + the first d2 matmuls
                    # give PE ~7us of work that needs only g, hiding the
                    # x0 transfer latency that used to stall startup.
                    nc.sync.dma_start(
                        g_sb, g.rearrange("(nt p) c -> p nt c", p=P)
                    )
                    nc.sync.dma_start(
                        bqk_sb,
                        bqkv[0 : 2 * H].rearrange("(mt p) -> p mt", p=P),
                    )
                    # p-state warm-up: the PE only reaches 2.4GHz after
                    # ~3us of CONTINUOUS execution; a few dependency-free
                    # dummy matmuls during the otherwise-idle g/x DMA
                    # window mean the real prologue starts at full clock
                    dumw = wstage.tile([P, SC], f32r)
                    nc.vector.memset(dumw.bitcast(f32), 0.0)
                    for _ in range(7):
                        psw = ps_o.tile([P, SC], f32, tag="pso")
                        nc.tensor.matmul(
                            psw,
                            lhsT=dumw[:, 0:P],
                            rhs=dumw,
                            start=True,
                            stop=True,
                        )
                    g2 = wstage.tile([P, NT, 3], f32)
                    nc.vector.tensor_mul(g2, g_sb, g_sb)
                    sq = wstage.tile([P, NT, 1], f32)
                    nc.vector.reduce_sum(sq, g2, axis=mybir.AxisListType.X)
                    Ag = wstage.tile([P, NT, 8], f32)
                    Ah = wstage.tile([P, NT, 8], f32)
                    nc.vector.memset(Ag, 0.0)
                    nc.vector.memset(Ah, 0.0)
                    nc.vector.tensor_copy(Ag[:, :, 0:3], g_sb)
                    nc.vector.tensor_copy(Ag[:, :, 3:4], sq)
                    nc.vector.memset(Ag[:, :, 4:5], 1.0)
                    nc.vector.tensor_scalar_mul(Ah[:, :, 0:3], g_sb, -2.0)
                    nc.vector.memset(Ah[:, :, 3:4], 1.0)
                    nc.vector.tensor_copy(Ah[:, :, 4:5], sq)
                    for nt in range(NT):
                        pt = ps_s.tile([P, SC], f32, tag="pss")
                        nc.tensor.transpose(pt[:8, :P], Ah[:, nt, :], ident)
                        nc.scalar.copy(
                            hT8[:, nt * P : (nt + 1) * P], pt[:8, :P]
                        )
                        pt2 = ps_d.tile([P, SC], f32, tag="psd")
                        nc.tensor.transpose(pt2[:8, :P], Ag[:, nt, :], ident)
                        nc.scalar.copy(
                            gT8[:, nt * P : (nt + 1) * P], pt2[:8, :P]
                        )
                    for _ in range(4):
                        emit_d2(*d2q.pop(0))
                    for qi, hh in enumerate(range(0, NT, NH)):
                        x_sb = xsb_pool.tile([P, NH, H], f32, tag="x_sb")
                        nc.sync.dma_start(x_sb, x_r[:, hh : hh + NH, :])
                        # Pool converts the group to bf16 (SBUF->SBUF);
                        # bf16 transposes run at 1 cycle/row and 4 of
                        # them pack one [P,SC] bf16 PSUM row evacuated by
                        # a single 2x-rate DVE copy into the strided xT
                        # destination
                        x_bf = xsb_pool.tile([P, NH, H], bf16, tag="x_bf")
                        if qi == 0:
                            # idle DVE converts the first group in halves
                            # so transposing starts ~1.5us sooner than
                            # waiting on the Pool engine
                            for nt in range(NH):
                                nc.vector.tensor_copy(
                                    x_bf[:, nt, :], x_sb[:, nt, :]
                                )
                        else:
                            nc.gpsimd.tensor_copy(x_bf, x_sb)
                        for wh in wq_after.get(qi, []):
                            emit_w_quarter(wh)
                        for nt in range(NH):
                            pt = ps_o.tile([P, SC], f32, tag="pso")
                            pbf = pt.bitcast(bf16)
                            for ht in range(HT):
                                nc.tensor.transpose(
                                    pbf[:, ht * P : (ht + 1) * P],
                                    x_bf[:, nt, ht * P : (ht + 1) * P],
                                    ident_bf,
                                )
                            nc.vector.tensor_copy(
                                xT[:, :, (hh + nt) * P : (hh + nt + 1) * P],
                                pbf[:, : HT * P],
                            )
                        for c, mt in blk_after.get(qi, []):
                            emit_qk_block(c, mt)

                nc.gpsimd.dma_start(bo_bc, bout.partition_broadcast(P))

            while d2q:
                emit_d2(*d2q.pop(0))
            emit_sqrt_exp(0)

            # ---- pipelined main loop over i-chunks ----
            y_r = y.rearrange("(nt p) h -> p nt h", p=P)
            UTs = {}
            # one persistent bank for the row-sum columns, halves
            # alternating by iteration parity (slice-level deps avoid a
            # cross-iteration WAR stall without a second bank)
            psl2 = ps_l.tile([P, 2, ITC], f32, tag="psl")

            for it in range(NC_ + 1):
                cur, prv, nxt = it, it - 1, it + 1
                E = Es.get(cur) if cur < NC_ else None
                if cur < NC_:
                    UTc = ut_pool.tile([P, NT, SC], bf16, tag="UT")
                    UTs[cur] = UTc
                    isl = slice(cur * SC, (cur + 1) * SC)
                if prv >= 0:
                    UTp = UTs.pop(prv)
                    # softmax row-sums for prv, in column form: tiny
                    # rhs=ones matmuls ride each attn@V group (same lhsT
                    # slices), so l_i lands as [P,1] PSUM columns with
                    # ~zero PE occupancy and no ACT wait; a per-column
                    # reciprocal feeds the fused finalize directly.
                    psl_c = psl2[:, prv % 2, :]
                    lc = et_pool.tile([P, ITC], f32, tag="lc")
                for jt in range(NT):
                    if cur < NC_:
                        jo = jt
                        jsl = slice(jo * P, (jo + 1) * P)
                        pss = ps_s.tile([P, SC], f32, tag="pss")
                        for kc in (0, 2):
                            nc.tensor.matmul(
                                pss,
                                lhsT=kT[:, kc : kc + 2, jsl],
                                rhs=qT[:, kc : kc + 2, isl],
                                start=(kc == 0),
                                stop=(kc == 2),
                                perf_mode=DR,
                            )
                        # score*E + expU right after QK: this is the
                        # iteration's critical path, so it goes first in
                        # the in-order DVE queue (clamps are not urgent
                        # and follow). NOT paired: the evacuation must
                        # start as soon as half jt lands or the scores
                        # pipeline stalls on the 2-bank ring.
                        if jt % 2 == 0:
                            et2 = et_pool.tile([P, 2, SC], f32, tag="et")
                        nc.vector.tensor_mul(
                            et2[:, jt % 2, :], pss, E[:, jo, :]
                        )
                        if jt % 2 == 1:
                            chain(
                                nc.scalar.activation(
                                    UTc[:, jo - 1 : jo + 1, :], et2, AF.Exp
                                )
                            )
                    if it == 0:
                        # v' natural [j, h] (bf16) as iteration-0 PE filler
                        psv = ps_o.tile([P, SC], f32, tag="pso")
                        for kc in range(HT):
                            nc.tensor.matmul(
                                psv,
                                lhsT=xT[:, kc, jt * P : (jt + 1) * P],
                                rhs=wqkv_bf[:, kc, 2 * H : 3 * H],
                                start=(kc == 0),
                                stop=(kc == HT - 1),
                            )
                        if jt % 2 == 0:
                            nc.scalar.copy(v_bf[:, jt, :], psv)
                        else:
                            nc.vector.tensor_copy(v_bf[:, jt, :], psv)
                    if prv >= 0:
                        # attn@V for prv in natural layout: i-tile-major,
                        # psy accumulates the output chunk directly
                        it4, sub = jt // SPB, jt % SPB
                        if sub == 0:
                            psy = ps_o.tile([P, SC], f32, tag="pso")
                        for jtt in range(sub * JPS, (sub + 1) * JPS):
                            ut_ap = UTp[:, jtt, it4 * P : (it4 + 1) * P]
                            nc.tensor.matmul(
                                psy,
                                lhsT=ut_ap,
                                rhs=v_bf[:, jtt, :],
                                start=(jtt == 0),
                                stop=(jtt == NT - 1),
                            )
                            nc.tensor.matmul(
                                psl_c[:, it4 : it4 + 1],
                                lhsT=ut_ap,
                                rhs=ones_bf,
                                start=(jtt == 0),
                                stop=(jtt == NT - 1),
                            )
                        if sub == SPB - 1:
                            # single fused finalize on DVE: with the sqrt
                            # back on ACT, ACT is the loaded engine and
                            # DVE has the slack
                            nc.vector.reciprocal(
                                lc[:, it4 : it4 + 1], psl_c[:, it4 : it4 + 1]
                            )
                            ysb = small.tile([P, H], f32, tag="ysb")
                            nc.vector.scalar_tensor_tensor(
                                ysb,
                                psy,
                                lc[:, it4 : it4 + 1],
                                bo_bc,
                                OP.mult,
                                OP.add,
                            )
                            nc.sync.dma_start(
                                y_r[:, prv * ITC + it4, :], ysb
                            )
                    if nxt < NC_:
                        # schedule the 4 symmetric-transpose j-tiles at
                        # the END steps: their PE transposes read the
                        # exp'ed E(cur), which the in-order ACT queue may
                        # still be finishing early in the iteration
                        if SYM:
                            # j-tiles whose rows lie in a RESIDENT earlier
                            # chunk (ring distance <= 2) are bf16
                            # transposes; the rest need the d2 pipeline.
                            # The contiguous sqrt+exp ACT block fires as
                            # soon as the last computed quad's clamp is
                            # in flight (same 2 table swaps anywhere in
                            # the iteration, but landing early unblocks
                            # the next iteration's first score*E mults).
                            eTjs = [
                                j for j in range(NT)
                                if j // ITC < nxt and nxt - j // ITC <= 2
                            ]
                            d2js = [j for j in range(NT) if j not in eTjs]
                            if nxt == 1:
                                d2js = []  # chunk 1's d2s rode the prologue
                            if jt < len(d2js):
                                emit_d2(nxt, d2js[jt])
                            elif jt >= NT - len(eTjs):
                                emit_eT(nxt, eTjs[jt - (NT - len(eTjs))], E)
                        else:
                            emit_d2(nxt, jt)
                if nxt < NC_:
                    emit_sqrt_exp(nxt)


def build_bass(n: int = 2048) -> bass.Bass:
    nc = bacc.Bacc(None, target_bir_lowering=False)
    x = nc.dram_tensor("x", [n, H], f32, kind="ExternalInput")[:, :]
    g = nc.dram_tensor("g", [n, 3], f32, kind="ExternalInput")[:, :]
    wqkv = nc.dram_tensor("w_qkv", [H, 3 * H], f32, kind="ExternalInput")[:, :]
    bqkv = nc.dram_tensor("b_qkv", [3 * H], f32, kind="ExternalInput")[:]
    bout = nc.dram_tensor("b_out", [H], f32, kind="ExternalInput")[:]
    y = nc.dram_tensor("y", [n, H], f32, kind="ExternalOutput")[:, :]
    with tile.TileContext(nc) as tc:
        _body(tc, n, x, g, wqkv, bqkv, bout, y)
    nc.finalize()
    return nc


_CACHED = {}


def _get_nc(n: int = 2048) -> bass.Bass:
    if n not in _CACHED:
        _CACHED[n] = build_bass(n)
    return _CACHED[n]


def _fold_wout(wqkv, bqkv, wout, bout):
    """Host-side fold of W_out into the v-projection (exact: the model
    is linear after the softmax and softmax rows sum to 1)."""
    wqkv_eff = np.array(wqkv, dtype=np.float32, copy=True)
    wv = wqkv[:, 2 * H :].astype(np.float64)
    wqkv_eff[:, 2 * H :] = (wv @ wout.astype(np.float64)).astype(np.float32)
    beff = (
        bqkv[2 * H :].astype(np.float64) @ wout.astype(np.float64)
        + bout.astype(np.float64)
    ).astype(np.float32)
    return wqkv_eff, beff


def kernel(**inputs) -> np.ndarray:
    from concourse.bass_utils import run_bass_kernel_spmd

    x = np.ascontiguousarray(inputs["x"], dtype=np.float32)
    g = np.ascontiguousarray(inputs["geometric_features"], dtype=np.float32)
    wqkv = np.ascontiguousarray(inputs["W_qkv"], dtype=np.float32)
    bqkv = np.ascontiguousarray(inputs["b_qkv"], dtype=np.float32)
    wout = np.ascontiguousarray(inputs["W_out"], dtype=np.float32)
    bout = np.ascontiguousarray(inputs["b_out"], dtype=np.float32)

    wqkv_eff, beff = _fold_wout(wqkv, bqkv, wout, bout)

    B, n, _ = x.shape
    nc = _get_nc(n)
    core_ids = list(range(B))
    in_maps = [
        {
            "x": np.ascontiguousarray(x[b]),
            "g": np.ascontiguousarray(g[b]),
            "w_qkv": wqkv_eff,
            "b_qkv": bqkv,
            "b_out": beff,
        }
        for b in range(B)
    ]
    res = run_bass_kernel_spmd(nc, in_maps, core_ids)
    return np.stack([res.results[b]["y"] for b in range(B)]).astype(np.float32)
